# revision 29
# baseline (speedup 1.0000x reference)
"""Trainium2 Bass kernel: MultiHeadAttention over [2, 512, 64, 64] images.

Sharding: 8 cores = (2 batches) x (4 head-pairs). Each core computes 2 of the
8 attention heads for one batch plus a partial output projection over its 128
input channels; the host sums the 4 partial projections per batch and adds the
bias terms that commute with the attention (bv rides through the softmax
weights exactly, bk is a per-query score shift the softmax is invariant to,
bp is a plain output offset).

Per-core schedule (all L=4096 positions), engine-balanced around the Scalar
(ACT) engine which is the hard bottleneck (exp of 33.5M scores at 1
elem/lane/cycle):
  QKV:  Q/K in [c=128, l] f16 (2 heads x 64 dk on partitions), Q pre-scaled
        by softmax_scale/16 (host folds it into Wq), V transposed on the PE
        into VT [s, c] f32r with interleaved ones columns.
  Attn: software-pipelined s-loop per 512-wide t-tile: STs are issued two
        tiles ahead in a flat (t,s) order so the PE never waits on the exp
        of tile s; exp runs split across ACT (cols [0,AW), scale=16) and a
        pair of custom DVE ops (deg-4 Taylor poly p(u) then p^16 by four
        squarings) for cols [AW,1024). The ones rows in VT accumulate the
        softmax denominators in PSUM rows 64/129 for free. The h1 AV
        stream runs 4 s-tiles behind h0 (PSUM accumulation order is free)
        so the first h1 write of a t-tile lands after the previous tile's
        av psum slot has drained.
  Tail: per t-tile normalize (approx-reciprocal + ones-outer-product
        broadcast in fp32 + multiply) and the 4 partial-Wp projections are
        deferred into the next t-tile's s-loop at fixed s slots with
        deprioritized scheduling so stale deps never stall the PE stream.
"""

import math
import numpy as np

B, C, HH, WW = 2, 512, 64, 64
L = HH * WW          # 4096
NH, DK = 8, 64
SCALE = 1.0 / math.sqrt(DK)
NCORES = 8

TT = 512             # t-tile width (columns per attention tile)
NT = L // TT         # 8 t-tiles
NS = L // 128        # 32 s-tiles
KT = C // 128        # 4 contraction tiles for projections

AW = 832 # exp columns [0,AW) on ACT, [AW,1024) on DVE custom op

_BUILT = {}
_EXP_OPS = None
DEBUG = False


def _register_exp_ops():
    """Register the two custom DVE ops used for the exp tail share.

    EXP_POLY4_ANT: p = 1 + u*(1 + u*(c2 + u*(c3 + u*c4)))   (8 ALU stages)
    EXP_SQ4_ANT:   out = ((p^2)^2)^2)^2 = p^16               (4 ALU stages)
    With u = score*scale/16 (|u| <~ 0.09 for this problem), p^16 = exp(16u)
    to ~1e-6 relative.
    """
    global _EXP_OPS
    if _EXP_OPS is not None:
        return _EXP_OPS
    import concourse.dve_ops as dvo
    from concourse.dve_spec import Spec, Src0, One, C0, C1, C2, sq, lower
    from concourse.dve_uop import DveOpSpec

    if "EXP_POLY4_ANT" in dvo._SUB_OPCODE_FOR_NAME:
        _EXP_OPS = (dvo.__dict__["EXP_POLY4_ANT"], dvo.__dict__["EXP_SQ4_ANT"])
        return _EXP_OPS

    h = Src0 * C0 + C1
    h = h * Src0 + C2
    h = h * Src0 + One
    p = h * Src0 + One
    spec_poly = Spec(
        body=p,
        reference=lambda in0, in1, s0, s1, imm2: 1.0
        + in0 * (1.0 + in0 * (imm2 + in0 * (s1 + in0 * s0))),
    )
    spec_sq4 = Spec(
        body=sq(sq(sq(sq(Src0)))),
        reference=lambda in0, in1, s0, s1, imm2: (((in0**2) ** 2) ** 2) ** 2,
    )

    ops = []
    for name, spec in (("EXP_POLY4_ANT", spec_poly), ("EXP_SQ4_ANT", spec_sq4)):
        shas = {}
        for ver in ("v3", "v4"):
            uops = lower(spec, ver=ver)
            shas[ver] = DveOpSpec(
                name=name, opcode=0, uops=uops, rd1_en=False
            ).sha(ver)
        op = dvo.DveOp(name, spec, subdim=False, uops_sha=shas)
        dvo.OPS.append(op)
        dvo.CUSTOM_DVE_SPECS[name] = spec
        dvo._SUB_OPCODE_FOR_NAME[name] = dvo._CUSTOM_DVE_ROW_BASE + len(dvo.OPS) - 1
        ops.append(op)
    _EXP_OPS = tuple(ops)
    return _EXP_OPS


def _build(l=L):
    import concourse.bacc as bacc
    import concourse.tile as tile
    import concourse.mybir as mybir
    from concourse.masks import make_identity
    from contextlib import ExitStack

    exp_poly, exp_sq4 = _register_exp_ops()

    nt = l // TT
    ns = l // 128
    f32 = mybir.dt.float32
    f16 = mybir.dt.float16
    f32r = mybir.dt.float32r
    Exp = mybir.ActivationFunctionType.Exp
    add = mybir.AluOpType.add
    mult = mybir.AluOpType.mult

    nc = bacc.Bacc("TRN2", target_bir_lowering=False, debug=False,
                   num_devices=NCORES)

    x = nc.dram_tensor("x", [C, l], f16, kind="ExternalInput").ap()
    # wpack: [128, 12*128 (wq|wk|wv kt-tiles) + 512 (wp)] fp16
    wpack = nc.dram_tensor("wpack", [128, 2048], f16, kind="ExternalInput").ap()
    bq = nc.dram_tensor("bq", [128, 1], f32, kind="ExternalInput").ap()
    out = nc.dram_tensor("out", [C, l], f32, kind="ExternalOutput").ap()
    dbg = {}
    if DEBUG:
        dbg["z0"] = nc.dram_tensor("dbg_z0", [1, TT], f32,
                                   kind="ExternalOutput").ap()
        dbg["rz0"] = nc.dram_tensor("dbg_rz0", [1, TT], f32,
                                    kind="ExternalOutput").ap()
        dbg["zbs"] = nc.dram_tensor("dbg_zbs", [128, TT], f32,
                                    kind="ExternalOutput").ap()
        dbg["ou"] = nc.dram_tensor("dbg_ou", [128, TT], f32,
                                   kind="ExternalOutput").ap()
        dbg["osb"] = nc.dram_tensor("dbg_osb", [128, TT], f16,
                                    kind="ExternalOutput").ap()
        dbg["e0"] = nc.dram_tensor("dbg_e0", [128, 2 * TT], f32,
                                   kind="ExternalOutput").ap()

    DVW = 1024 - AW  # dve exp column count

    with tile.TileContext(nc) as tc, ExitStack() as ctx:
        persist = ctx.enter_context(tc.tile_pool(name="persist", bufs=1))
        e_pool = ctx.enter_context(tc.tile_pool(name="e", bufs=6))
        p_pool = ctx.enter_context(tc.tile_pool(name="p", bufs=2))
        o_pool = ctx.enter_context(tc.tile_pool(name="o", bufs=2))
        z_pool = ctx.enter_context(tc.tile_pool(name="z", bufs=2))
        res_pool = ctx.enter_context(tc.tile_pool(name="res", bufs=3))

        # ---- weights: one packed fp16 block, 4 chunked DMAs ----
        wall = persist.tile([128, 2048], f16, tag="wall")
        for c8 in range(8):
            nc.sync.dma_start(out=wall[:, c8 * 256:(c8 + 1) * 256],
                              in_=wpack[:, c8 * 256:(c8 + 1) * 256])
        w_r = wall[:, 0:1536].rearrange("p (i kt c) -> p i kt c", i=3, kt=KT)
        wp_r = wall[:, 1536:2048]

        bq_sb = persist.tile([128, 1], f32, tag="bq")
        nc.sync.dma_start(out=bq_sb, in_=bq)

        ident = persist.tile([128, 128], f16, tag="ident")
        make_identity(nc, ident)

        # [1,128] masked-ones stationaries: m0 selects psum rows 0:64,
        # m1 rows 64:128 -- lets both heads' denominator broadcasts land at
        # dst partition base 0 (f32r matmuls reject dst base 64)
        mask_f = persist.tile([1, 256], f32, tag="maskf")
        nc.vector.memset(mask_f, 0.0)
        nc.vector.memset(mask_f[:, 0:64], 1.0)
        nc.vector.memset(mask_f[:, 192:256], 1.0)
        mask_r = persist.tile([1, 256], f32r, tag="maskr")
        nc.vector.tensor_copy(mask_r, mask_f)

        # ---- persistent activations ----
        x_sb = persist.tile([128, KT, l], f16, tag="xsb")
        for n in range(l // TT):
            nh = 2 if n == 0 else 1
            hw_ = TT // nh
            for kt in range(KT):
                for hh in range(nh):
                    c0 = n * TT + hh * hw_
                    nc.gpsimd.dma_start(
                        out=x_sb[:, kt, c0:c0 + hw_],
                        in_=x[kt * 128:(kt + 1) * 128, c0:c0 + hw_])
        q_sb = persist.tile([128, l], f16, tag="q")
        k_sb = persist.tile([128, l], f16, tag="k")
        vt_sb = persist.tile([128, ns, 130], f32r, tag="vt")
        # interleaved ones columns -> softmax denominator rows in AV psum
        ones_col = persist.tile([128, ns, 1], f32, tag="onescol")
        nc.vector.memset(ones_col, 1.0)
        nc.vector.tensor_copy(vt_sb[:, :, 64:65], ones_col)
        nc.vector.tensor_copy(vt_sb[:, :, 129:130], ones_col)

        # ================= QKV projections + V transpose =================
        with ExitStack() as qctx:
            qkv_ps = qctx.enter_context(
                tc.tile_pool(name="qkvps", bufs=2, space="PSUM"))
            v_pool = qctx.enter_context(tc.tile_pool(name="vsb", bufs=2))

            for n in range(nt):
                nsl = slice(n * TT, (n + 1) * TT)
                x_r = x_sb[:, :, nsl]

                q_ps = qkv_ps.tile([128, TT], f32, tag="qps")
                k_ps = qkv_ps.tile([128, TT], f32, tag="kps")
                v_ps = qkv_ps.tile([128, TT], f32, tag="vps")
                for wi, ps in enumerate((q_ps, k_ps, v_ps)):
                    for kt in range(KT):
                        nc.tensor.matmul(ps, w_r[:, wi, kt, :], x_r[:, kt, :],
                                         start=(kt == 0), stop=(kt == KT - 1))

                nc.vector.tensor_scalar(q_sb[:, nsl], q_ps,
                                        bq_sb, None, add)
                nc.scalar.copy(k_sb[:, nsl], k_ps)
                v_sb = v_pool.tile([128, TT], f16, tag="v")
                nc.scalar.copy(v_sb, v_ps)

                # transpose V tile: 4 PE transposes -> [s, c] in psum
                tp = qkv_ps.tile([128, TT], f16, tag="tp")
                for j in range(4):
                    nc.tensor.transpose(tp[:, j * 128:(j + 1) * 128],
                                        v_sb[:, j * 128:(j + 1) * 128], ident)
                tp_v = tp.rearrange("p (j c) -> p j c", j=4)
                ssl = slice(4 * n, 4 * n + 4)
                nc.vector.tensor_copy(vt_sb[:, ssl, 0:64], tp_v[:, :, 0:64])
                nc.vector.tensor_copy(vt_sb[:, ssl, 65:129], tp_v[:, :, 64:128])

        # ========================= attention =========================
        with ExitStack() as actx:
            st_pool = actx.enter_context(
                tc.tile_pool(name="stps", bufs=2, space="PSUM"))
            av_pool = actx.enter_context(
                tc.tile_pool(name="avps", bufs=3, space="PSUM"))
            pr_pool = actx.enter_context(
                tc.tile_pool(name="prps", bufs=1, space="PSUM"))

            # Deferred tail work from the previous t-tile. The reciprocal +
            # denominator-broadcast chain is issued right at the t-boundary
            # (rcp first in the DVE queue, zbmm after two ST prefetches in
            # the PE queue); normalize muls and the partial projections are
            # spread over the next t-tile's s-loop at fixed slots, PE work
            # issued first within an iteration so stale deps never block
            # younger PE instructions in the in-order queue.
            def make_tail(av0, av1, tsl, dbg_t=False, split_dma=False):
                rz = z_pool.tile([1, 2 * TT], f32, tag="rz")
                rz_r = z_pool.tile([1, 2 * TT], f32r, tag="rzr")
                zb = pr_pool.tile([128, TT], f32, tag="pp")
                ou = z_pool.tile([128, TT], f32, tag="ou")
                o_sb = o_pool.tile([128, TT], f16, tag="o")

                zr0 = z_pool.tile([1, TT], f32, tag="zr0")
                zr1 = z_pool.tile([1, TT], f32, tag="zr1")

                def rcp():
                    # custom-DVE ops drop the partition offset of their in0,
                    # so stage the z rows to partition-0 tiles first
                    nc.vector.tensor_copy(zr0, av0[64:65, :])
                    nc.vector.reciprocal_approx_fast(out=rz[:, 0:TT], in_=zr0)
                    nc.vector.tensor_copy(zr1, av1[64:65, :])
                    nc.vector.reciprocal_approx_fast(out=rz[:, TT:2 * TT],
                                                     in_=zr1)
                    nc.vector.tensor_copy(rz_r, rz)
                    if dbg_t:
                        nc.sync.dma_start(out=dbg["z0"], in_=zr0)
                        nc.sync.dma_start(out=dbg["rz0"], in_=rz[:, 0:TT])

                def zbmm():
                    # zb[0:64] = rz0 bcast, zb[64:128] = rz1 bcast via two
                    # accumulating masked-ones f32r matmuls, both dst base 0
                    nc.tensor.matmul(zb, mask_r[:, 0:128],
                                     rz_r[:, 0:TT], start=True, stop=False)
                    nc.tensor.matmul(zb, mask_r[:, 128:256],
                                     rz_r[:, TT:2 * TT], start=False,
                                     stop=True)

                def ev0():
                    nc.vector.tensor_copy(ou[0:64, :], av0[0:64, :])

                def ev1():
                    nc.vector.tensor_copy(ou[64:128, :], av1[0:64, :])

                def mul():
                    nc.vector.tensor_mul(o_sb, ou, zb)
                    if dbg_t:
                        nc.sync.dma_start(out=dbg["ou"], in_=ou)
                        nc.sync.dma_start(out=dbg["osb"], in_=o_sb)

                work = [(0, False, ev0), (1, False, ev1), (2, True, zbmm),
                        (3, False, mul)]
                for i, ot in enumerate(range(KT)):
                    pp = pr_pool.tile([128, TT], f32, tag="pp")

                    def mmfn(pp=pp, ot=ot):
                        nc.tensor.matmul(pp, wp_r[:, ot * 128:(ot + 1) * 128],
                                         o_sb, start=True, stop=True)

                    def cpfn(pp=pp, ot=ot):
                        res = res_pool.tile([128, TT], f32, tag="res")
                        nc.vector.tensor_copy(res, pp)
                        if split_dma:
                            q = TT // 4
                            for c4 in range(4):
                                nc.gpsimd.dma_start(
                                    out=out[ot * 128:(ot + 1) * 128,
                                            tsl.start + c4 * q:
                                            tsl.start + (c4 + 1) * q],
                                    in_=res[:, c4 * q:(c4 + 1) * q])
                        else:
                            nc.gpsimd.dma_start(
                                out=out[ot * 128:(ot + 1) * 128, tsl],
                                in_=res)

                    work.append((8 + 4 * i, True, mmfn))
                    work.append((10 + 4 * i, False, cpfn))
                return rcp, work

            def issue_st_g(g):
                t2, s2 = divmod(g, ns)
                tsl2 = slice(t2 * TT, (t2 + 1) * TT)
                ssl = slice(s2 * 128, (s2 + 1) * 128)
                st_ps = st_pool.tile([128, 2 * TT], f32, tag="st")
                nc.tensor.matmul(st_ps[:, 0:TT], k_sb[0:64, ssl],
                                 q_sb[0:64, tsl2], start=True, stop=True)
                nc.tensor.matmul(st_ps[:, TT:2 * TT], k_sb[64:128, ssl],
                                 q_sb[64:128, tsl2], start=True, stop=True)
                return st_ps

            st_tiles = {}
            tail_rcp, tail = None, []
            for t in range(nt):
                tsl = slice(t * TT, (t + 1) * TT)
                av0 = av_pool.tile([128, TT], f32, tag="av")
                av1 = av_pool.tile([128, TT], f32, tag="av")

                h1q, e_tiles = [], {}
                if t == 0:
                    for g0 in range(2):
                        st_tiles[g0] = issue_st_g(g0)
                if tail_rcp is not None:
                    tail_rcp()

                for s in range(ns):
                    for slot, is_pe, fn in tail:
                        if slot == s and is_pe:
                            with tc.high_priority(offset=-700):
                                fn()
                    g = t * ns + s + 2
                    if g < nt * ns:
                        st_tiles[g] = issue_st_g(g)
                    st_ps = st_tiles.pop(t * ns + s)

                    e_sb = e_pool.tile([128, 2 * TT], f32r, tag="e")
                    nc.scalar.activation(e_sb[:, 0:AW], st_ps[:, 0:AW], Exp,
                                         scale=16.0)
                    p_sb = p_pool.tile([128, DVW], f32, tag="p")
                    nc.vector._custom_dve(exp_poly, out=p_sb,
                                          in0=st_ps[:, AW:2 * TT],
                                          s0=1.0 / 24.0, s1=1.0 / 6.0,
                                          imm2=0.5)
                    nc.vector._custom_dve(exp_sq4,
                                          out=e_sb[:, AW:2 * TT], in0=p_sb)

                    if DEBUG and t == 0 and s == 0:
                        nc.sync.dma_start(out=dbg["e0"],
                                          in_=e_sb.bitcast(f32))
                    nc.tensor.matmul(av0[0:65, :], vt_sb[:, s, 0:65],
                                     e_sb[:, 0:TT], start=(s == 0),
                                     stop=(s == ns - 1))
                    # h1 AV stream runs ~4 s-tiles behind h0 (PSUM accumulate
                    # order is free) so the first h1 write of a t-tile lands
                    # after the previous tile's av slot is drained
                    e_tiles[s] = e_sb
                    h1q.append(s)
                    npop = 2 if 4 <= s < 8 else (1 if s >= 8 else 0)
                    for _ in range(npop):
                        j = h1q.pop(0)
                        ej = e_tiles.pop(j)
                        nc.tensor.matmul(av1[0:65, :], vt_sb[:, j, 65:130],
                                         ej[:, TT:2 * TT], start=(j == 0),
                                         stop=(j == ns - 1))

                    for slot, is_pe, fn in tail:
                        if slot == s and not is_pe:
                            with tc.high_priority(offset=-700):
                                fn()
                tail_rcp, tail = make_tail(av0, av1, tsl,
                                           dbg_t=(DEBUG and t == 0),
                                           split_dma=(t == nt - 1))

            # epilogue: flush the final t-tile's tail work
            tail_rcp()
            for slot, is_pe, fn in sorted(tail, key=lambda w: w[0]):
                fn()

    nc.compile()
    return nc


def _get_nc(l=L):
    if l not in _BUILT:
        _BUILT[l] = _build(l)
    return _BUILT[l]


def _shard_inputs(x, Wq, bq, Wkv, bkv, Wp, bp, l=L):
    x = np.asarray(x, dtype=np.float32)
    Wq = np.asarray(Wq, dtype=np.float32)
    bq = np.asarray(bq, dtype=np.float32)
    Wkv = np.asarray(Wkv, dtype=np.float32)
    bkv = np.asarray(bkv, dtype=np.float32)
    Wp = np.asarray(Wp, dtype=np.float32)
    bp = np.asarray(bp, dtype=np.float32)

    s16 = SCALE / 16.0
    in_maps = []
    for core in range(NCORES):
        b, hp = divmod(core, 4)
        sl = slice(hp * 128, (hp + 1) * 128)
        vsl = slice(C + hp * 128, C + (hp + 1) * 128)
        wq_t = (Wq[sl, :] * s16).T.astype(np.float16)      # [C, 128]
        wk_t = Wkv[sl, :].T.astype(np.float16)
        wv_t = Wkv[vsl, :].T.astype(np.float16)
        wp_t = Wp[:, sl].T.astype(np.float16)               # [128, C]
        wpack = np.empty((128, 2048), dtype=np.float16)
        for i, w in enumerate((wq_t, wk_t, wv_t)):
            for kt in range(KT):
                wpack[:, (i * KT + kt) * 128:(i * KT + kt + 1) * 128] = \
                    w[kt * 128:(kt + 1) * 128, :]
        wpack[:, 1536:2048] = wp_t
        m = {
            "x": np.ascontiguousarray(x[b].reshape(C, l).astype(np.float16)),
            "wpack": np.ascontiguousarray(wpack),
            "bq": np.ascontiguousarray(
                (bq[sl] * s16).reshape(128, 1).astype(np.float32)),
        }
        in_maps.append(m)
    return in_maps


def _host_const(Wkv, bkv, Wp, bp):
    """Bias terms folded out of the kernel: out += Wp @ bv + bp (exact)."""
    Wkv = np.asarray(Wkv, dtype=np.float64)
    bkv = np.asarray(bkv, dtype=np.float64)
    Wp = np.asarray(Wp, dtype=np.float64)
    bp = np.asarray(bp, dtype=np.float64)
    bv = bkv[C:]
    return (Wp @ bv + bp).astype(np.float32)


def _run(in_maps, l=L, trace=False):
    from concourse.bass_utils import run_bass_kernel_spmd
    nc = _get_nc(l)
    return run_bass_kernel_spmd(nc, in_maps, core_ids=list(range(NCORES)),
                                trace=trace)


def kernel(x, Wq, bq, Wkv, bkv, Wp, bp):
    in_maps = _shard_inputs(x, Wq, bq, Wkv, bkv, Wp, bp)
    res = _run(in_maps)
    outs = [res.results[i]["out"].astype(np.float32) for i in range(NCORES)]
    cv = _host_const(Wkv, bkv, Wp, bp)[None, :, None]
    y = np.stack([outs[0] + outs[1] + outs[2] + outs[3],
                  outs[4] + outs[5] + outs[6] + outs[7]]) + cv
    return np.ascontiguousarray(y.reshape(B, C, HH, WW), dtype=np.float32)
